# revision 34
# baseline (speedup 1.0000x reference)
"""Trainium2 Bass kernel for nn_CNN_88098369175781.

Model: x[1,1,18,T=262144] -> wavA=x[...,0,:], eeg=x[...,1:17,:], wavB=x[...,17,:]
  wav streams: proj(1->16, pointwise) -> diagonal sinc filter bank (15 taps,
  pad 7) -> conv(16->10, 9 taps) + bias -> relu -> global max-pool.
  eeg stream:  conv(16->10, 9 taps) + bias -> relu -> global max-pool.
  concat -> sigmoid FC(30->30) -> sigmoid FC(30->2).

Device decomposition (validated vs reference in numpy):
  * Each wav stream's three linear stages fuse into ONE 1->10 channel, 23-tap
    conv on the zero-padded raw wav signal (weights precomposed on host).
  * Bias/relu commute past the global max (bias is constant over time;
    max(relu(h)) = relu(max(h))), so the device only computes convs + maxima.
  * Convs run on the tensor engine via a polyphase formulation:
      eeg:  time phases r in [0,8), outputs (o, dt in [0,8)) => M=80,
            contraction (c,r) => K=128, 2 accumulating matmuls (u-groups).
      wav:  time phases v in [0,12), outputs (o, dt in [0,12)) => M=120,
            contraction (v,q in [0,3)) => K=36, single matmul per tile
            (the q-replication is materialized host-side).
  * Operands are fp16 (PSUM accumulation stays fp32): fp32 matmuls run as two
    HW passes on trn2, fp16 single-pass -- and DMA bytes halve. Validated
    rel err ~4e-6 vs the fp32 reference.
  * Max-reduction is split across engines: the vector engine reduces eeg PSUM
    directly (fp32); the scalar engine casts wav PSUM to fp16 in SBUF and the
    vector engine reduces those at the 2-byte 2x mode.
  * 8 cores split the time axis (overlapping chunks; overlap is free for max).
  * Host combines per-core maxima and runs the tiny FC head.
"""

import os
import numpy as np

T = 262144
NOUT = T - 8            # 262136 valid conv output positions
NCORES = 8
KLEN = 15
SIGMA = 0.005

EEG_NCOL = 4096         # eeg matmul columns per core (8 outputs each)
EEG_COLS = EEG_NCOL + 1  # phase row length (g=1 needs one extra column)
WAV_NCOL = 2731         # wav matmul columns per core (12 outputs each)
EEG_TC = 8 * EEG_NCOL   # 32768 eeg outputs per core
WAV_TC = 12 * WAV_NCOL  # 32772 wav outputs per core

_NC_CACHE = {}
LAST_RESULT = None      # BassKernelResults of the most recent device run


# --------------------------------------------------------------------------
# host-side weight precompute
# --------------------------------------------------------------------------

def _sinc_rows(mu):
    """Diagonal rows of the reference's sinc_kernel: [16, 15] float64."""
    k = np.linspace(-1.0, 1.0, KLEN)
    kk = (k[None, :] - np.asarray(mu, np.float64)[:, None]) / SIGMA
    nos = np.sum(np.abs(kk) < 1e-5, axis=1)
    kk = np.where((nos >= 0.5)[:, None], kk - 5e-5, kk)
    return np.sin(np.pi * kk) / (np.pi * kk)


def _composite_wav_weights(mu, proj_w, conv_w_i):
    """Fused 1->10ch 23-tap kernel E[o, s] (float64)."""
    krn = _sinc_rows(mu)                                  # [16,15]
    a = np.asarray(proj_w, np.float64)[:, 0, 0]           # [16]
    W = np.asarray(conv_w_i, np.float64)                  # [10,16,9]
    E = np.zeros((10, 23))
    for j in range(9):
        E[:, j:j + 15] += np.einsum('oc,cm->om', W[:, :, j] * a[None, :], krn)
    return E


def _eeg_lhsT(W1):
    """[128, 160]: cols g*80+(o*8+dt); row c*8+r; val W1[o,c,8g+r-dt]."""
    W1 = np.asarray(W1, np.float64)
    out = np.zeros((128, 160))
    g, c, r, o, dt = np.meshgrid(np.arange(2), np.arange(16), np.arange(8),
                                 np.arange(10), np.arange(8), indexing='ij')
    j = 8 * g + r - dt
    valid = (j >= 0) & (j < 9)
    out[(c * 8 + r)[valid], (g * 80 + o * 8 + dt)[valid]] = \
        W1[o[valid], c[valid], np.clip(j[valid], 0, 8)]
    return out.astype(np.float32)


def _wav_lhsT(E):
    """[36, 120]: row v*3+q, col o*12+dt, val E[o, 12q+v-dt]."""
    out = np.zeros((36, 120))
    v, q, o, dt = np.meshgrid(np.arange(12), np.arange(3), np.arange(10),
                              np.arange(12), indexing='ij')
    s = 12 * q + v - dt
    valid = (s >= 0) & (s < 23)
    out[(v * 3 + q)[valid], (o * 12 + dt)[valid]] = E[o[valid], np.clip(s[valid], 0, 22)]
    return out.astype(np.float32)


# --------------------------------------------------------------------------
# host-side per-core input slicing
# --------------------------------------------------------------------------

def _core_starts(k):
    return (min(k * 32767, NOUT - EEG_TC), min(k * 32767, NOUT - WAV_TC))


def _eeg_phases(eeg, k):
    """[128, 4097]: row c*8+r, col m = eeg[c, s_e + 8m + r]."""
    s_e, _ = _core_starts(k)
    v = eeg[:, s_e:s_e + 8 * EEG_COLS]                  # [16, 32776]
    p = v.reshape(16, EEG_COLS, 8).transpose(0, 2, 1)   # [16,8,4097]
    return p.reshape(128, EEG_COLS)


def _wav_phases(w_pad, k):
    """[36, 2731]: row v*3+q, col n = w_pad[s_w + 12(n+q) + v]."""
    _, s_w = _core_starts(k)
    sl = w_pad[s_w:s_w + 12 * (WAV_NCOL + 2)]
    y = sl.reshape(WAV_NCOL + 2, 12).T                  # y[v,m] = sl[12m+v]
    out = np.empty((36, WAV_NCOL), dtype=w_pad.dtype)
    for q in range(3):
        out[q::3, :] = y[:, q:q + WAV_NCOL]
    return out


# --------------------------------------------------------------------------
# bass kernel
# --------------------------------------------------------------------------

def _build_nc():
    import concourse.bacc as bacc
    import concourse.tile as tile
    import concourse.mybir as mybir

    f32 = mybir.dt.float32
    f16 = mybir.dt.float16
    nc = bacc.Bacc("TRN2", target_bir_lowering=False, debug=False,
                   num_devices=NCORES)

    eegP = nc.dram_tensor("eegP", [128, EEG_COLS], f16, kind="ExternalInput")
    wavP = nc.dram_tensor("wavP", [36, 2 * WAV_NCOL], f16, kind="ExternalInput")
    wts = nc.dram_tensor("wts", [128, 400], f16, kind="ExternalInput")
    out = nc.dram_tensor("out", [128, 10], f16, kind="ExternalOutput")

    N_ECHUNK = 4                 # eeg input loaded in 4 column chunks
    ECHUNK = 1024                # chunk j covers cols [1024j, 1024j+1025)
    N_WARM = 6                   # dummy matmuls to warm the PE clock gate

    with tile.TileContext(nc) as tc:
        with (
            tc.tile_pool(name="sb", bufs=1) as sb,
            tc.tile_pool(name="ps", bufs=4, space="PSUM") as psp,
        ):
            # PE warmup: dummy matmuls on a zeroed scratch tile keep the PE
            # busy while the first input DMAs land, so the HAM clock-gate
            # opens (1.2 -> 2.4 GHz) before the real matmuls start.
            scr = sb.tile([128, 512], f16, tag="scr")
            nc.gpsimd.memset(scr[:], 0.0)
            wps = psp.tile([120, 1024], f32, tag="ps", name="wps")
            for _ in range(N_WARM):
                nc.tensor.matmul(wps[0:80, 0:512], scr[:, 0:80], scr[:],
                                 start=True, stop=True)

            # input DMAs spread over three issue engines: descriptor
            # generation costs ~0.7us per dma_start and serializes per engine
            echunks = [sb.tile([128, ECHUNK + 1], f16, tag=f"eegchunk{j}",
                               name=f"eegchunk{j}") for j in range(N_ECHUNK)]
            wts_t = sb.tile([128, 400], f16, tag="wts")
            wav_t = sb.tile([36, 2 * WAV_NCOL], f16, tag="wav")
            nc.sync.dma_start(wts_t[:], wts[:])
            nc.sync.dma_start(echunks[0][:], eegP[:, 0:ECHUNK + 1])
            nc.sync.dma_start(echunks[1][:], eegP[:, ECHUNK:2 * ECHUNK + 1])
            nc.sync.dma_start(wav_t[:], wavP[:])
            nc.sync.dma_start(echunks[2][:], eegP[:, 2 * ECHUNK:3 * ECHUNK + 1])
            nc.sync.dma_start(echunks[3][:], eegP[:, 3 * ECHUNK:4 * ECHUNK + 1])
            wE_t = wts_t[:, 0:160]

            # merged fp16 output tile; host finishes the max over columns
            # cols: 0=eeg p0, 1=eeg p2, 2=wavA tail, 3=wavB tail (fp32 path),
            #       4=eeg p1, 5=eeg p3, 6,7=wavA p0,p1, 8,9=wavB p0,p1 (fp16)
            out16 = sb.tile([128, 10], f16, tag="out16")
            nc.gpsimd.memset(out16[:], 0.0)
            # fp32 maxima landing tile for the PSUM-direct reduces
            mF = sb.tile([120, 4], f32, tag="mF")
            nc.gpsimd.memset(mF[:], 0.0)
            # fp16 staging tiles for the ACT-evacuated psum pairs
            stg = [sb.tile([120, 1024], f16, tag=f"stg{i}", name=f"stg{i}")
                   for i in range(4)]

            X = mybir.AxisListType.X
            Copy = mybir.ActivationFunctionType.Copy

            # PSUM evacuation is split: ACT casts pairs to fp16 SBUF
            # ((N+352)/1.2 per op) while DVE direct-reduces the others from
            # PSUM (1 elem/cycle), then reduces the fp16 staged tiles.

            # eeg: pairs 0,2 -> DVE direct; pairs 1,3 -> ACT cast
            for p in range(4):
                ch = echunks[p]
                ps = psp.tile([120, 1024], f32, tag="ps", name=f"pse{p}")
                for g in range(2):
                    for j in range(2):
                        lo = j * 512
                        nc.tensor.matmul(ps[0:80, lo:lo + 512],
                                         wE_t[:, 80 * g:80 * g + 80],
                                         ch[:, lo + g:lo + g + 512],
                                         start=(g == 0), stop=(g == 1))
                if p % 2 == 0:
                    nc.vector.reduce_max(mF[0:80, p // 2:p // 2 + 1],
                                         ps[0:80, :], axis=X)
                else:
                    nc.scalar.activation(stg[p // 2][0:80, :], ps[0:80, :], Copy)
            nc.vector.reduce_max(out16[0:80, 4:5], stg[0][0:80, :], axis=X)
            nc.vector.reduce_max(out16[0:80, 5:6], stg[1][0:80, :], axis=X)

            # wav: per stream, pairs 0,1 -> ACT cast + fp16 reduce; tail (683
            # cols) -> DVE direct
            for si in range(2):
                s0, s1 = (stg[2], stg[3]) if si == 0 else (stg[0], stg[1])
                for p in range(3):
                    ps = psp.tile([120, 1024], f32, tag="ps", name=f"psw{si}{p}")
                    for j in range(2):
                        n0 = si * WAV_NCOL + (2 * p + j) * 512
                        nn = min(512, (si + 1) * WAV_NCOL - n0)
                        nc.tensor.matmul(ps[:, j * 512:j * 512 + nn],
                                         wts_t[0:36, 160 + 120 * si:280 + 120 * si],
                                         wav_t[:, n0:n0 + nn],
                                         start=True, stop=True)
                    if p < 2:
                        nc.scalar.activation((s0 if p == 0 else s1)[:], ps[:], Copy)
                    else:
                        nc.vector.reduce_max(mF[:, 2 + si:3 + si],
                                             ps[:, 0:683], axis=X)
                nc.vector.reduce_max(out16[0:120, 6 + 2 * si:7 + 2 * si],
                                     s0[:], axis=X)
                nc.vector.reduce_max(out16[0:120, 7 + 2 * si:8 + 2 * si],
                                     s1[:], axis=X)

            # cast the fp32 maxima into the merged fp16 output tile
            nc.vector.tensor_copy(out16[0:120, 0:4], mF[:])

            nc.sync.dma_start(out[:], out16[:])

    nc.compile()
    return nc


def _get_nc():
    if "nc" not in _NC_CACHE:
        _NC_CACHE["nc"] = _build_nc()
    return _NC_CACHE["nc"]


# --------------------------------------------------------------------------
# entry point
# --------------------------------------------------------------------------

def _prepare_in_maps(x, mu, projA_w, projB_w, conv_w):
    x = np.asarray(x, np.float32)
    eeg = np.ascontiguousarray(x[0, 0, 1:17, :]).astype(np.float16)
    zt = np.zeros(64, np.float32)
    w_padA = np.concatenate([np.zeros(7, np.float32), x[0, 0, 0, :], zt]
                            ).astype(np.float16)
    w_padB = np.concatenate([np.zeros(7, np.float32), x[0, 0, 17, :], zt]
                            ).astype(np.float16)

    conv_w = np.asarray(conv_w)
    E_A = _composite_wav_weights(mu, projA_w, conv_w[0])
    E_B = _composite_wav_weights(mu, projB_w, conv_w[2])
    wts_np = np.zeros((128, 400), np.float16)
    wts_np[:, 0:160] = _eeg_lhsT(conv_w[1])
    wts_np[0:36, 160:280] = _wav_lhsT(E_A)
    wts_np[0:36, 280:400] = _wav_lhsT(E_B)

    in_maps = []
    for k in range(NCORES):
        wavp = np.concatenate([_wav_phases(w_padA, k), _wav_phases(w_padB, k)],
                              axis=1)
        in_maps.append({
            "eegP": np.ascontiguousarray(_eeg_phases(eeg, k)),
            "wavP": np.ascontiguousarray(wavp),
            "wts": wts_np,
        })
    return in_maps


def _head(percore, conv_b, fc1_w, fc1_b, fc2_w, fc2_b):
    m = percore.max(axis=0).astype(np.float64)
    eeg_o = m[0:80].reshape(10, 8).max(axis=1)
    wavA_o = m[80:200].reshape(10, 12).max(axis=1)
    wavB_o = m[200:320].reshape(10, 12).max(axis=1)
    conv_b = np.asarray(conv_b, np.float64)
    f = np.concatenate([np.maximum(wavA_o + conv_b[0], 0.0),
                        np.maximum(eeg_o + conv_b[1], 0.0),
                        np.maximum(wavB_o + conv_b[2], 0.0)])
    h = 1.0 / (1.0 + np.exp(-(f @ np.asarray(fc1_w, np.float64).T
                              + np.asarray(fc1_b, np.float64))))
    o = 1.0 / (1.0 + np.exp(-(h @ np.asarray(fc2_w, np.float64).T
                              + np.asarray(fc2_b, np.float64))))
    return o[None, :].astype(np.float32)


def _percore_from_out(arr):
    """Device 'out' [128,10] fp16 -> flat [320] (eeg 80, wavA 120, wavB 120).

    eeg partial maxima in cols 0,1,4,5; wavA in 2,6,7; wavB in 3,8,9."""
    arr = np.asarray(arr, np.float32)
    return np.concatenate([arr[0:80, [0, 1, 4, 5]].max(axis=1),
                           arr[0:120, [2, 6, 7]].max(axis=1),
                           arr[0:120, [3, 8, 9]].max(axis=1)])


def kernel(x, mu, projA_w, projB_w, conv_w, conv_b, fc1_w, fc1_b, fc2_w, fc2_b):
    global LAST_RESULT
    in_maps = _prepare_in_maps(x, mu, projA_w, projB_w, conv_w)
    nc = _get_nc()

    if os.environ.get("KERNEL_USE_SIM"):
        # sim mode for correctness checking without hardware
        from concourse.bass_interp import CoreSim
        percore = np.zeros((NCORES, 320), np.float32)
        for k in range(NCORES):
            sim = CoreSim(nc)
            for name, arr in in_maps[k].items():
                sim.tensor(name)[:] = arr
            sim.simulate()
            percore[k] = _percore_from_out(sim.tensor("out"))
    else:
        from concourse.bass_utils import run_bass_kernel_spmd
        trace = bool(os.environ.get("KERNEL_TRACE"))
        res = run_bass_kernel_spmd(nc, in_maps, list(range(NCORES)),
                                   trace=trace)
        LAST_RESULT = res
        percore = np.stack([_percore_from_out(res.results[k]["out"])
                            for k in range(NCORES)])

    return _head(percore, conv_b, fc1_w, fc1_b, fc2_w, fc2_b)


# revision 37
# speedup vs baseline: 1.0806x; 1.0806x over previous
"""Trainium2 Bass kernel for nn_CNN_88098369175781.

Model: x[1,1,18,T=262144] -> wavA=x[...,0,:], eeg=x[...,1:17,:], wavB=x[...,17,:]
  wav streams: proj(1->16, pointwise) -> diagonal sinc filter bank (15 taps,
  pad 7) -> conv(16->10, 9 taps) + bias -> relu -> global max-pool.
  eeg stream:  conv(16->10, 9 taps) + bias -> relu -> global max-pool.
  concat -> sigmoid FC(30->30) -> sigmoid FC(30->2).

Device decomposition (validated vs reference in numpy):
  * Each wav stream's three linear stages fuse into ONE 1->10 channel, 23-tap
    conv on the zero-padded raw wav signal (weights precomposed on host).
  * Bias/relu commute past the global max (bias is constant over time;
    max(relu(h)) = relu(max(h))), so the device only computes convs + maxima.
  * Convs run on the tensor engine via a polyphase formulation:
      eeg:  time phases r in [0,8), outputs (o, dt in [0,8)) => M=80,
            contraction (c,r) => K=128, 2 accumulating matmuls (u-groups).
      wav:  time phases v in [0,12), outputs (o, dt in [0,12)) => M=120,
            contraction (v,q in [0,3)) => K=36, single matmul per tile
            (the q-replication is materialized host-side).
  * Operands are fp16 (PSUM accumulation stays fp32): fp32 matmuls run as two
    HW passes on trn2, fp16 single-pass -- and DMA bytes halve. Validated
    rel err ~4e-6 vs the fp32 reference.
  * Max-reduction is split across engines: the vector engine reduces eeg PSUM
    directly (fp32); the scalar engine casts wav PSUM to fp16 in SBUF and the
    vector engine reduces those at the 2-byte 2x mode.
  * 8 cores split the time axis (overlapping chunks; overlap is free for max).
  * Host combines per-core maxima and runs the tiny FC head.
"""

import os
import numpy as np

T = 262144
NOUT = T - 8            # 262136 valid conv output positions
NCORES = 8
KLEN = 15
SIGMA = 0.005

EEG_NCOL = 4096         # eeg matmul columns per core (8 outputs each)
EEG_COLS = EEG_NCOL + 1  # phase row length (g=1 needs one extra column)
WAV_NCOL = 2731         # wav matmul columns per core (12 outputs each)
EEG_TC = 8 * EEG_NCOL   # 32768 eeg outputs per core
WAV_TC = 12 * WAV_NCOL  # 32772 wav outputs per core

_NC_CACHE = {}
LAST_RESULT = None      # BassKernelResults of the most recent device run


# --------------------------------------------------------------------------
# host-side weight precompute
# --------------------------------------------------------------------------

def _sinc_rows(mu):
    """Diagonal rows of the reference's sinc_kernel: [16, 15] float64."""
    k = np.linspace(-1.0, 1.0, KLEN)
    kk = (k[None, :] - np.asarray(mu, np.float64)[:, None]) / SIGMA
    nos = np.sum(np.abs(kk) < 1e-5, axis=1)
    kk = np.where((nos >= 0.5)[:, None], kk - 5e-5, kk)
    return np.sin(np.pi * kk) / (np.pi * kk)


def _composite_wav_weights(mu, proj_w, conv_w_i):
    """Fused 1->10ch 23-tap kernel E[o, s] (float64)."""
    krn = _sinc_rows(mu)                                  # [16,15]
    a = np.asarray(proj_w, np.float64)[:, 0, 0]           # [16]
    W = np.asarray(conv_w_i, np.float64)                  # [10,16,9]
    E = np.zeros((10, 23))
    for j in range(9):
        E[:, j:j + 15] += np.einsum('oc,cm->om', W[:, :, j] * a[None, :], krn)
    return E


def _eeg_lhsT(W1):
    """[128, 160]: cols g*80+(o*8+dt); row c*8+r; val W1[o,c,8g+r-dt]."""
    W1 = np.asarray(W1, np.float64)
    out = np.zeros((128, 160))
    g, c, r, o, dt = np.meshgrid(np.arange(2), np.arange(16), np.arange(8),
                                 np.arange(10), np.arange(8), indexing='ij')
    j = 8 * g + r - dt
    valid = (j >= 0) & (j < 9)
    out[(c * 8 + r)[valid], (g * 80 + o * 8 + dt)[valid]] = \
        W1[o[valid], c[valid], np.clip(j[valid], 0, 8)]
    return out.astype(np.float32)


def _wav_lhsT(E):
    """[36, 120]: row v*3+q, col o*12+dt, val E[o, 12q+v-dt]."""
    out = np.zeros((36, 120))
    v, q, o, dt = np.meshgrid(np.arange(12), np.arange(3), np.arange(10),
                              np.arange(12), indexing='ij')
    s = 12 * q + v - dt
    valid = (s >= 0) & (s < 23)
    out[(v * 3 + q)[valid], (o * 12 + dt)[valid]] = E[o[valid], np.clip(s[valid], 0, 22)]
    return out.astype(np.float32)


# --------------------------------------------------------------------------
# host-side per-core input slicing
# --------------------------------------------------------------------------

def _core_starts(k):
    return (min(k * 32767, NOUT - EEG_TC), min(k * 32767, NOUT - WAV_TC))


def _eeg_phases(eeg, k):
    """[128, 4097]: row c*8+r, col m = eeg[c, s_e + 8m + r]."""
    s_e, _ = _core_starts(k)
    v = eeg[:, s_e:s_e + 8 * EEG_COLS]                  # [16, 32776]
    p = v.reshape(16, EEG_COLS, 8).transpose(0, 2, 1)   # [16,8,4097]
    return p.reshape(128, EEG_COLS)


def _wav_phases(w_pad, k):
    """[36, 2731]: row v*3+q, col n = w_pad[s_w + 12(n+q) + v]."""
    _, s_w = _core_starts(k)
    sl = w_pad[s_w:s_w + 12 * (WAV_NCOL + 2)]
    y = sl.reshape(WAV_NCOL + 2, 12).T                  # y[v,m] = sl[12m+v]
    out = np.empty((36, WAV_NCOL), dtype=w_pad.dtype)
    for q in range(3):
        out[q::3, :] = y[:, q:q + WAV_NCOL]
    return out


# --------------------------------------------------------------------------
# bass kernel
# --------------------------------------------------------------------------

def _build_nc():
    import concourse.bacc as bacc
    import concourse.tile as tile
    import concourse.mybir as mybir

    f32 = mybir.dt.float32
    f16 = mybir.dt.float16
    nc = bacc.Bacc("TRN2", target_bir_lowering=False, debug=False,
                   num_devices=NCORES)

    eegP = nc.dram_tensor("eegP", [128, EEG_COLS], f16, kind="ExternalInput")
    wavP = nc.dram_tensor("wavP", [36, 2 * WAV_NCOL], f16, kind="ExternalInput")
    wts = nc.dram_tensor("wts", [128, 400], f16, kind="ExternalInput")
    out = nc.dram_tensor("out", [128, 10], f16, kind="ExternalOutput")

    N_ECHUNK = 2                 # eeg input loaded in 2 column chunks
    ECHUNK = 2048                # chunk j covers cols [2048j, 2048j+2049)
    N_WARM = 6                   # dummy matmuls to warm the PE clock gate

    with tile.TileContext(nc) as tc:
        with (
            tc.tile_pool(name="sb", bufs=1) as sb,
            tc.tile_pool(name="ps", bufs=4, space="PSUM") as psp,
        ):
            # PE warmup: dummy matmuls on a zeroed scratch tile keep the PE
            # busy while the first input DMAs land, so the HAM clock-gate
            # opens (1.2 -> 2.4 GHz) before the real matmuls start.
            scr = sb.tile([128, 512], f16, tag="scr")
            nc.gpsimd.memset(scr[:], 0.0)
            wps = psp.tile([120, 1024], f32, tag="ps", name="wps")
            for _ in range(N_WARM):
                nc.tensor.matmul(wps[0:80, 0:512], scr[:, 0:80], scr[:],
                                 start=True, stop=True)

            # input DMAs spread over three issue engines: descriptor
            # generation costs ~0.7us per dma_start and serializes per engine
            echunks = [sb.tile([128, ECHUNK + 1], f16, tag=f"eegchunk{j}",
                               name=f"eegchunk{j}") for j in range(N_ECHUNK)]
            wts_t = sb.tile([128, 400], f16, tag="wts")
            wav_t = sb.tile([36, 2 * WAV_NCOL], f16, tag="wav")
            nc.scalar.dma_start(wts_t[:], wts[:])
            nc.sync.dma_start(echunks[0][:], eegP[:, 0:ECHUNK + 1])
            nc.sync.dma_start(echunks[1][:], eegP[:, ECHUNK:2 * ECHUNK + 1])
            nc.scalar.dma_start(wav_t[:], wavP[:])
            wE_t = wts_t[:, 0:160]

            # merged fp16 output tile; host finishes the max over columns
            # cols: 0=eeg p0, 1=eeg p2, 2=wavA tail, 3=wavB tail (fp32 path),
            #       4=eeg p1, 5=eeg p3, 6,7=wavA p0,p1, 8,9=wavB p0,p1 (fp16)
            out16 = sb.tile([128, 10], f16, tag="out16")
            nc.gpsimd.memset(out16[:], 0.0)
            # fp32 maxima landing tile for the PSUM-direct reduces
            mF = sb.tile([120, 4], f32, tag="mF")
            nc.gpsimd.memset(mF[:], 0.0)
            # fp16 staging tiles for the ACT-evacuated psum pairs
            stg = [sb.tile([120, 1024], f16, tag=f"stg{i}", name=f"stg{i}")
                   for i in range(4)]

            X = mybir.AxisListType.X
            Copy = mybir.ActivationFunctionType.Copy

            # PSUM evacuation is split: ACT casts pairs to fp16 SBUF
            # ((N+352)/1.2 per op) while DVE direct-reduces the others from
            # PSUM (1 elem/cycle), then reduces the fp16 staged tiles.

            # eeg: pairs 0,2 -> DVE direct; pairs 1,3 -> ACT cast
            for p in range(4):
                ch = echunks[p // 2]
                base = (p % 2) * 1024
                ps = psp.tile([120, 1024], f32, tag="ps", name=f"pse{p}")
                for g in range(2):
                    for j in range(2):
                        lo = j * 512
                        nc.tensor.matmul(ps[0:80, lo:lo + 512],
                                         wE_t[:, 80 * g:80 * g + 80],
                                         ch[:, base + lo + g:base + lo + g + 512],
                                         start=(g == 0), stop=(g == 1))
                if p % 2 == 0:
                    nc.vector.reduce_max(mF[0:80, p // 2:p // 2 + 1],
                                         ps[0:80, :], axis=X)
                else:
                    nc.scalar.activation(stg[p // 2][0:80, :], ps[0:80, :], Copy)
            nc.vector.reduce_max(out16[0:80, 4:5], stg[0][0:80, :], axis=X)
            nc.vector.reduce_max(out16[0:80, 5:6], stg[1][0:80, :], axis=X)

            # wav: per stream, pairs 0,1 -> ACT cast + fp16 reduce; tail (683
            # cols) -> DVE direct
            for si in range(2):
                s0, s1 = (stg[2], stg[3]) if si == 0 else (stg[0], stg[1])
                for p in range(3):
                    ps = psp.tile([120, 1024], f32, tag="ps", name=f"psw{si}{p}")
                    for j in range(2):
                        n0 = si * WAV_NCOL + (2 * p + j) * 512
                        nn = min(512, (si + 1) * WAV_NCOL - n0)
                        nc.tensor.matmul(ps[:, j * 512:j * 512 + nn],
                                         wts_t[0:36, 160 + 120 * si:280 + 120 * si],
                                         wav_t[:, n0:n0 + nn],
                                         start=True, stop=True)
                    if p < 2:
                        nc.scalar.activation((s0 if p == 0 else s1)[:], ps[:], Copy)
                    else:
                        nc.vector.reduce_max(mF[:, 2 + si:3 + si],
                                             ps[:, 0:683], axis=X)
                nc.vector.reduce_max(out16[0:120, 6 + 2 * si:7 + 2 * si],
                                     s0[:], axis=X)
                nc.vector.reduce_max(out16[0:120, 7 + 2 * si:8 + 2 * si],
                                     s1[:], axis=X)

            # cast the fp32 maxima into the merged fp16 output tile
            nc.vector.tensor_copy(out16[0:120, 0:4], mF[:])

            nc.sync.dma_start(out[:], out16[:])

    nc.compile()
    return nc


def _get_nc():
    if "nc" not in _NC_CACHE:
        _NC_CACHE["nc"] = _build_nc()
    return _NC_CACHE["nc"]


# --------------------------------------------------------------------------
# entry point
# --------------------------------------------------------------------------

def _prepare_in_maps(x, mu, projA_w, projB_w, conv_w):
    x = np.asarray(x, np.float32)
    eeg = np.ascontiguousarray(x[0, 0, 1:17, :]).astype(np.float16)
    zt = np.zeros(64, np.float32)
    w_padA = np.concatenate([np.zeros(7, np.float32), x[0, 0, 0, :], zt]
                            ).astype(np.float16)
    w_padB = np.concatenate([np.zeros(7, np.float32), x[0, 0, 17, :], zt]
                            ).astype(np.float16)

    conv_w = np.asarray(conv_w)
    E_A = _composite_wav_weights(mu, projA_w, conv_w[0])
    E_B = _composite_wav_weights(mu, projB_w, conv_w[2])
    wts_np = np.zeros((128, 400), np.float16)
    wts_np[:, 0:160] = _eeg_lhsT(conv_w[1])
    wts_np[0:36, 160:280] = _wav_lhsT(E_A)
    wts_np[0:36, 280:400] = _wav_lhsT(E_B)

    in_maps = []
    for k in range(NCORES):
        wavp = np.concatenate([_wav_phases(w_padA, k), _wav_phases(w_padB, k)],
                              axis=1)
        in_maps.append({
            "eegP": np.ascontiguousarray(_eeg_phases(eeg, k)),
            "wavP": np.ascontiguousarray(wavp),
            "wts": wts_np,
        })
    return in_maps


def _head(percore, conv_b, fc1_w, fc1_b, fc2_w, fc2_b):
    m = percore.max(axis=0).astype(np.float64)
    eeg_o = m[0:80].reshape(10, 8).max(axis=1)
    wavA_o = m[80:200].reshape(10, 12).max(axis=1)
    wavB_o = m[200:320].reshape(10, 12).max(axis=1)
    conv_b = np.asarray(conv_b, np.float64)
    f = np.concatenate([np.maximum(wavA_o + conv_b[0], 0.0),
                        np.maximum(eeg_o + conv_b[1], 0.0),
                        np.maximum(wavB_o + conv_b[2], 0.0)])
    h = 1.0 / (1.0 + np.exp(-(f @ np.asarray(fc1_w, np.float64).T
                              + np.asarray(fc1_b, np.float64))))
    o = 1.0 / (1.0 + np.exp(-(h @ np.asarray(fc2_w, np.float64).T
                              + np.asarray(fc2_b, np.float64))))
    return o[None, :].astype(np.float32)


def _percore_from_out(arr):
    """Device 'out' [128,10] fp16 -> flat [320] (eeg 80, wavA 120, wavB 120).

    eeg partial maxima in cols 0,1,4,5; wavA in 2,6,7; wavB in 3,8,9."""
    arr = np.asarray(arr, np.float32)
    return np.concatenate([arr[0:80, [0, 1, 4, 5]].max(axis=1),
                           arr[0:120, [2, 6, 7]].max(axis=1),
                           arr[0:120, [3, 8, 9]].max(axis=1)])


def kernel(x, mu, projA_w, projB_w, conv_w, conv_b, fc1_w, fc1_b, fc2_w, fc2_b):
    global LAST_RESULT
    in_maps = _prepare_in_maps(x, mu, projA_w, projB_w, conv_w)
    nc = _get_nc()

    if os.environ.get("KERNEL_USE_SIM"):
        # sim mode for correctness checking without hardware
        from concourse.bass_interp import CoreSim
        percore = np.zeros((NCORES, 320), np.float32)
        for k in range(NCORES):
            sim = CoreSim(nc)
            for name, arr in in_maps[k].items():
                sim.tensor(name)[:] = arr
            sim.simulate()
            percore[k] = _percore_from_out(sim.tensor("out"))
    else:
        from concourse.bass_utils import run_bass_kernel_spmd
        trace = bool(os.environ.get("KERNEL_TRACE"))
        res = run_bass_kernel_spmd(nc, in_maps, list(range(NCORES)),
                                   trace=trace)
        LAST_RESULT = res
        percore = np.stack([_percore_from_out(res.results[k]["out"])
                            for k in range(NCORES)])

    return _head(percore, conv_b, fc1_w, fc1_b, fc2_w, fc2_b)
